# revision 27
# baseline (speedup 1.0000x reference)
"""Causal self-attention (B=2, T=2048, C=1024, 16 heads) on 8 TRN2 NeuronCores.

Sharding: core = b*4 + hg (b data-parallel over batch, hg tensor-parallel over
head groups of 4 heads). Each core computes QKV for its 4 heads, causal
attention, and a partial output projection (its 256 rows of w_proj); the host
sums the 4 partials per batch element and adds b_proj once.

v4 design (fp32r baseline ~197us, v2 ~187us, v3 ~172.6us):
- bf16 storage + matmul operands (fp32 PSUM accumulation).
- S matmuls row-tiled: K=64 per head, two heads run CONCURRENTLY in PE row
  groups (0,0)/(64,0).
- exp on ScalarE (~86us) is the pacing engine; QKV/proj matmuls are spliced
  between S/AV pairs at single-matmul granularity. Fillers live in two
  queues: QKV chains are deadline-bound (must be emitted before the next
  block's attention reads them -> hard-drained at block boundaries), proj
  chains are soft and flow into the late blocks where exp-latency demand is
  highest. proj of block 2 is held back entirely to cover the final
  normalize window.
- Tile hazard tracking follows emission order coarsely, so order of python
  emission is load-bearing throughout.
- b_proj is added on the host (free), so proj PSUM->SBUF moves are pure
  copies; the tail moves run on the then-idle ScalarE.
- Input DMAs are few big multi-dim descriptors (Sync issues each in ~600ns),
  ordered so the first attention block's dependencies land first (wa column
  layout is [q01|k01|q23|k23|v] to make that prefix contiguous).
"""
import numpy as np
from collections import deque
from contextlib import ExitStack

import concourse.bass as bass
import concourse.tile as tile
from concourse import bacc, mybir
from concourse.bass_utils import run_bass_kernel_spmd

F32 = mybir.dt.float32
BF16 = mybir.dt.bfloat16
AF = mybir.ActivationFunctionType

B, T, C = 2, 2048, 1024
N_CORES = 8
KT = 8              # contraction tiles over C (1024/128)
NTQ = 4             # T blocks of 512 (query blocks)
SCALE = 1.0 / 8.0   # 1/sqrt(HEAD_DIM)
WAC = 768           # fused weight cols per k-chunk
VW = 65             # v_ext cols per head: [v(64) | 1]
CO_OFF = {0: 0, 2: 128, 1: 256, 3: 384}  # wa col offset per qk co tile

COST_QK = 213.0
COST_V = 107.0
COST_PROJ = 213.0
STEP_CREDIT = 450.0

_cached_nc = None


def _build():
    nc = bacc.Bacc("TRN2", target_bir_lowering=False, debug=False,
                   enable_asserts=True, num_devices=N_CORES)
    xt = nc.dram_tensor("xt", [C, T], BF16, kind="ExternalInput").ap()
    wa = nc.dram_tensor("wa", [C, WAC], BF16, kind="ExternalInput").ap()
    bqk = nc.dram_tensor("bqk", [128, 4], F32, kind="ExternalInput").ap()
    bvbc = nc.dram_tensor("bvbc", [128, 256], BF16, kind="ExternalInput").ap()
    wp = nc.dram_tensor("wp", [256, C], BF16, kind="ExternalInput").ap()
    tri2 = nc.dram_tensor("tri2", [128, 256], BF16, kind="ExternalInput").ap()
    y = nc.dram_tensor("y", [T, C], BF16, kind="ExternalOutput").ap()

    with tile.TileContext(nc) as tc, ExitStack() as ctx:
        big = ctx.enter_context(tc.tile_pool(name="big", bufs=1))
        work = ctx.enter_context(tc.tile_pool(name="work", bufs=2))
        psum = ctx.enter_context(tc.tile_pool(name="psum", bufs=1, space="PSUM"))

        # ---- persistent SBUF tensors (split per block for precise hazards) ----
        xt_sb = big.tile([128, KT * T], BF16, tag="xt")
        wa_sb = big.tile([128, KT * WAC], BF16, tag="wa")
        wp_sb = big.tile([128, 2 * C], BF16, tag="wp")
        qk_t = [[big.tile([128, 512], BF16, tag=f"qk{co}_{tq}", name=f"qk{co}_{tq}")
                 for tq in range(NTQ)] for co in range(4)]
        v_t = [big.tile([128, 4 * VW], BF16, tag=f"v{t16}", name=f"v{t16}")
               for t16 in range(16)]
        attn_t = [[big.tile([128, 512], BF16, tag=f"at{j}_{tq}", name=f"at{j}_{tq}")
                   for tq in range(NTQ)] for j in range(2)]
        bqk_sb = big.tile([128, 4], F32, tag="bqk")
        bvbc_sb = big.tile([128, 256], BF16, tag="bvbc")
        tri2_sb = big.tile([128, 256], BF16, tag="tri2")

        # scratch for PE warm-up matmuls (memset first so they unblock ASAP)
        scratch = big.tile([128, 512], BF16, tag="scratch")
        nc.gpsimd.memset(scratch[:], 0.0)

        # ones columns of v_ext (d=64 of each head slot); no input deps
        for t16 in range(16):
            ones_view = v_t[t16][:].rearrange("p (h d) -> p h d", d=VW)[:, :, 64:VW]
            nc.gpsimd.memset(ones_view, 1.0)

        # ---- input DMAs ----
        # Issue order is the critical path: the Sync engine issues one
        # descriptor per ~0.6-2us, so the first QKV chain's pieces go first
        # in fine granules and the tiny consts (needed only ~10us later)
        # are demoted behind them.
        wa3d = wa.rearrange("(k p) c -> p k c", p=128)
        wa3s = wa_sb[:].rearrange("p (k c) -> p k c", c=WAC)
        xt3d = xt.rearrange("(k p) t -> p k t", p=128)
        xt3s = xt_sb[:].rearrange("p (k t) -> p k t", t=T)
        QT = T // 4
        nc.sync.dma_start(wa3s[:, 0:4, 0:128], wa3d[:, 0:4, 0:128])    # q01 a
        nc.sync.dma_start(xt3s[:, 0:4, 0:QT], xt3d[:, 0:4, 0:QT])
        nc.sync.dma_start(wa3s[:, 4:KT, 0:128], wa3d[:, 4:KT, 0:128])  # q01 b
        nc.sync.dma_start(xt3s[:, 4:KT, 0:QT], xt3d[:, 4:KT, 0:QT])
        nc.sync.dma_start(wa3s[:, :, 128:256], wa3d[:, :, 128:256])    # k01
        nc.sync.dma_start(bqk_sb[:], bqk[:])
        nc.sync.dma_start(tri2_sb[:], tri2[:])
        nc.sync.dma_start(bvbc_sb[:], bvbc[:])
        nc.sync.dma_start(wa3s[:, :, 512:WAC], wa3d[:, :, 512:WAC])    # v
        nc.sync.dma_start(wa3s[:, :, 256:512], wa3d[:, :, 256:512])    # q23|k23
        nc.sync.dma_start(xt3s[:, :, QT:2 * QT], xt3d[:, :, QT:2 * QT])
        wp3d = wp.rearrange("(k p) c -> p k c", p=128)
        wp3s = wp_sb[:].rearrange("p (k c) -> p k c", c=C)
        nc.sync.dma_start(wp3s[:], wp3d[:])
        nc.sync.dma_start(xt3s[:, :, 2 * QT:3 * QT], xt3d[:, :, 2 * QT:3 * QT])
        nc.sync.dma_start(xt3s[:, :, 3 * QT:T], xt3d[:, :, 3 * QT:T])

        # PE warm-up: dummy matmuls with no DMA deps run from ~6.6us while
        # the inputs stream, flipping HAM to 2.4GHz before real work arrives
        # (the HAM needs ~3.4us of sustained PE activity; without these the
        # first ~7us of real matmuls run at half clock). Results unread.
        for i in range(14):
            dp = psum.tile([128, 512], F32, tag="mm", bufs=2, name=f"warm{i}")
            nc.tensor.matmul(dp[:], scratch[:, 0:128], scratch[:],
                             start=True, stop=True)

        # ---- chain builders: lists of (cost_ns, emit_fn) ----
        def qk_chain(co, tq):
            st = {}

            def step(k):
                def f():
                    if k == 0:
                        st["p"] = psum.tile([128, 512], F32, tag="mm", bufs=2,
                                            name=f"qk{co}_{tq}")
                    nc.tensor.matmul(
                        st["p"][:],
                        wa_sb[:, k * WAC + CO_OFF[co]: k * WAC + CO_OFF[co] + 128],
                        xt_sb[:, k * T + tq * 512: k * T + (tq + 1) * 512],
                        start=(k == 0), stop=(k == KT - 1))
                    if k == KT - 1:
                        nc.vector.tensor_scalar_add(
                            qk_t[co][tq][:], st["p"][:], bqk_sb[:, co:co + 1])
                return (COST_QK, f)
            return [step(k) for k in range(KT)]

        def v_chain(t16):
            st = {}

            def step(k):
                def f():
                    if k == 0:
                        st["p"] = psum.tile([128, 256], F32, tag="mm", bufs=2,
                                            name=f"v{t16}")
                    nc.tensor.matmul(
                        st["p"][:],
                        xt_sb[:, k * T + t16 * 128: k * T + (t16 + 1) * 128],
                        wa_sb[:, k * WAC + 512: (k + 1) * WAC],
                        start=(k == 0), stop=(k == KT - 1))
                    if k == KT - 1:
                        out3 = v_t[t16][:].rearrange("p (h d) -> p h d", d=VW)[:, :, 0:64]
                        in3 = st["p"][:].rearrange("p (h d) -> p h d", d=64)
                        b3 = bvbc_sb[:].rearrange("p (h d) -> p h d", d=64)
                        nc.vector.tensor_add(out3, in3, b3)
                return (COST_V, f)
            return [step(k) for k in range(KT)]

        def proj_chain(t16, n, move="v", ptag="mm"):
            st = {}

            def step(kc):
                def f():
                    if kc == 0:
                        st["p"] = psum.tile([128, 512], F32, tag=ptag, bufs=2,
                                            name=f"pr{t16}_{n}")
                    nc.tensor.matmul(
                        st["p"][:],
                        attn_t[kc][t16 // 4][:, (t16 % 4) * 128: (t16 % 4 + 1) * 128],
                        wp_sb[:, kc * C + n * 512: kc * C + (n + 1) * 512],
                        start=(kc == 0), stop=(kc == 1))
                    if kc == 1:
                        ysb = work.tile([128, 512], BF16, tag="y", bufs=4)
                        if move == "s":
                            nc.scalar.activation(ysb[:], st["p"][:], AF.Copy)
                        else:
                            nc.vector.tensor_copy(ysb[:], st["p"][:])
                        nc.sync.dma_start(
                            y[t16 * 128:(t16 + 1) * 128, n * 512:(n + 1) * 512], ysb[:])
                return (COST_PROJ, f)
            return [step(kc) for kc in range(2)]

        # ---- two filler queues: deadline-bound QKV, soft proj ----
        qkv_q = deque()
        proj_q = deque()
        carry = [0.0]

        def pull(budget):
            carry[0] = min(carry[0] + budget, 1400.0)
            while True:
                q = qkv_q if qkv_q else proj_q
                if not q or q[0][0] > carry[0]:
                    break
                cost, fn = q.popleft()
                fn()
                carry[0] -= cost

        def drain_qkv():
            while qkv_q:
                qkv_q.popleft()[1]()
            carry[0] = 0.0

        def run_now(steps):
            for _, fn in steps:
                fn()

        # ---- attention for one head pair (heads 2j, 2j+1) over one tq block ----
        def pair_attn(j, tqb, prefill=None):
            nkt = 4 * (tqb + 1)
            av_a = psum.tile([VW, 512], F32, tag="av", bufs=2, name=f"av{j}{tqb}a")
            av_b = psum.tile([VW, 512], F32, tag="av", bufs=2, name=f"av{j}{tqb}b")
            s_t, e_t = {}, {}

            def emit_S(kt):
                s = psum.tile([128, 1024], F32, tag="s", bufs=2)
                m = kt - 4 * tqb
                c0s = m * 128 if (m > 0 and tqb > 0) else 0
                for half in range(2):
                    nc.tensor.matmul(
                        s[:, half * 512 + c0s: (half + 1) * 512],
                        qk_t[2 + j][kt // 4][half * 64:(half + 1) * 64,
                                             (kt % 4) * 128: (kt % 4 + 1) * 128],
                        qk_t[j][tqb][half * 64:(half + 1) * 64, c0s:512],
                        start=True, stop=True, tile_position=(64 * half, 0))
                s_t[kt] = s

            def emit_exp(kt):
                e = work.tile([128, 1024], BF16, tag="e", bufs=3)
                s = s_t.pop(kt)
                m = kt - 4 * tqb
                if m >= 2 and tqb > 0:
                    # masked prefix of each half is never read by AV: skip it
                    c0 = m * 128
                    for half in range(2):
                        sl = slice(half * 512 + c0, (half + 1) * 512)
                        nc.scalar.activation(e[:, sl], s[:, sl], AF.Exp, scale=SCALE)
                else:
                    nc.scalar.activation(e[:], s[:], AF.Exp, scale=SCALE)
                if m >= 0:
                    c0 = m * 128
                    e3 = e[:].rearrange("p (h q) -> p h q", q=512)[:, :, c0:c0 + 128]
                    t3 = tri2_sb[:].rearrange("p (h q) -> p h q", q=128)
                    nc.vector.tensor_mul(e3, e3, t3)
                e_t[kt] = e

            def emit_AV(kt):
                m = kt - 4 * tqb
                c0 = m * 128 if m > 0 else 0
                e = e_t.pop(kt)
                for half, av in ((0, av_a), (1, av_b)):
                    h = 2 * j + half
                    nc.tensor.matmul(
                        av[:, c0:512],
                        v_t[kt][:, h * VW: (h + 1) * VW],
                        e[:, half * 512 + c0: (half + 1) * 512],
                        start=(kt == 0), stop=(kt == nkt - 1))

            emit_S(0)
            emit_exp(0)
            for kt in range(nkt):
                if kt + 1 < nkt:
                    emit_S(kt + 1)
                if kt == 0:
                    # AV(0) must wait for the PREVIOUS pair's normalize to
                    # free the av psum ring (~4.5us chain) -> cover the gap
                    # with independent filler before AV(0) enters the queue
                    pull(3200.0)
                emit_AV(kt)
                if kt + 1 < nkt:
                    emit_exp(kt + 1)
                pull(STEP_CREDIT)

            # reserve work emitted BEFORE the normalize fills the PE during it
            # (anything emitted after is hazard-ordered behind the last mul)
            if prefill:
                run_now(prefill)
                pull(1500.0)

            # normalize: attn = av[0:64] * 1/av[64]; den copies split across
            # DVE/ScalarE so both heads' chains start immediately
            for half, av in ((0, av_a), (1, av_b)):
                den = work.tile([1, 512], F32, tag="den", bufs=2)
                if half:
                    nc.scalar.activation(den[:], av[64:VW, :], AF.Copy)
                else:
                    nc.vector.tensor_copy(den[:], av[64:VW, :])
                recipf = work.tile([1, 512], F32, tag="recip", bufs=2)
                nc.vector.reciprocal_approx_fast(recipf[:], den[:])
                bcs = work.tile([64, 512], F32, tag="bcs", bufs=2)
                nc.gpsimd.partition_broadcast(bcs[:], recipf[:])
                nc.vector.tensor_mul(
                    attn_t[j][tqb][half * 64:(half + 1) * 64, :],
                    av[0:64, :], bcs[:])

        # ---- schedule ----
        # upfront QKV for tq block 0 (dense PE work during the DMA-bound start)
        run_now(qk_chain(0, 0))
        run_now(qk_chain(2, 0))
        for t16 in range(4):
            run_now(v_chain(t16))
        run_now(qk_chain(1, 0))
        run_now(qk_chain(3, 0))

        for tqb in range(NTQ):
            nxt = tqb + 1
            if nxt < NTQ:
                qkv_q.extend(qk_chain(0, nxt))
                qkv_q.extend(qk_chain(2, nxt))
                for t16 in range(4 * nxt, 4 * nxt + 4):
                    qkv_q.extend(v_chain(t16))
            pair_attn(0, tqb)
            if nxt < NTQ:
                qkv_q.extend(qk_chain(1, nxt))
                qkv_q.extend(qk_chain(3, nxt))
            if tqb < NTQ - 1:
                pair_attn(1, tqb)
            else:
                # proj(block 2) held in reserve: emitted before the final
                # normalize so its matmuls execute during that window
                reserve = []
                for i, (t16, n) in enumerate(
                        [(t, n) for t in range(8, 12) for n in range(2)]):
                    reserve.extend(proj_chain(t16, n, move=("s", "v")[i % 2],
                                              ptag=("mm", "s")[i % 2]))
                pair_attn(1, tqb, prefill=reserve)
            drain_qkv()
            # proj for this block becomes soft filler (block 2 held for tail)
            if tqb < 2:
                for t16 in range(4 * tqb, 4 * tqb + 4):
                    for n in range(2):
                        proj_q.extend(proj_chain(t16, n))

        while proj_q:
            proj_q.popleft()[1]()
        # tail: proj of the last tq block, PSUM->SBUF moves on idle ScalarE,
        # psum slots rotated over the now-free s/av rings for pipeline depth
        for i, (t16, n) in enumerate(
                [(t, n) for t in range(12, 16) for n in range(2)]):
            run_now(proj_chain(t16, n, move=("s", "v")[i % 2],
                               ptag=("mm", "s", "av")[i % 3]))

    nc.compile()
    return nc


def _get_nc():
    global _cached_nc
    if _cached_nc is None:
        _cached_nc = _build()
    return _cached_nc


def make_in_maps(x, w_attn, b_attn, w_proj, b_proj):
    BF = mybir.dt.np(BF16)
    x = np.asarray(x, np.float32)
    w_attn = np.asarray(w_attn, np.float32)
    b_attn = np.asarray(b_attn, np.float32)
    w_proj = np.asarray(w_proj, np.float32)
    tri = np.triu(np.ones((128, 128), np.float32))
    tri2 = np.tile(tri, (1, 2)).astype(BF)
    in_maps = []
    for core in range(N_CORES):
        b, hg = core // 4, core % 4
        cs = slice(hg * 256, (hg + 1) * 256)
        wq = w_attn[:, cs]
        wk = w_attn[:, 1024 + hg * 256:1024 + (hg + 1) * 256]
        wv = w_attn[:, 2048 + hg * 256:2048 + (hg + 1) * 256]
        # col layout [q01|k01|q23|k23|v] so the startup DMA prefix is contiguous
        wa = np.ascontiguousarray(np.concatenate(
            [wq[:, 0:128], wk[:, 0:128], wq[:, 128:256], wk[:, 128:256], wv],
            axis=1)).astype(BF)
        bqk_vec = np.concatenate(
            [b_attn[cs], b_attn[1024 + hg * 256:1024 + (hg + 1) * 256]])
        in_maps.append({
            "xt": np.ascontiguousarray(x[b].T).astype(BF),
            "wa": wa,
            "bqk": np.ascontiguousarray(bqk_vec.reshape(4, 128).T).astype(np.float32),
            "bvbc": np.broadcast_to(
                b_attn[2048 + hg * 256:2048 + (hg + 1) * 256], (128, 256)).astype(BF),
            "wp": np.ascontiguousarray(w_proj[cs, :]).astype(BF),
            "tri2": tri2,
        })
    return in_maps


def kernel(x, w_attn, b_attn, w_proj, b_proj):
    in_maps = make_in_maps(x, w_attn, b_attn, w_proj, b_proj)
    nc = _get_nc()
    res = run_bass_kernel_spmd(nc, in_maps, core_ids=list(range(N_CORES)))
    y = np.zeros((B, T, C), np.float32)
    for core in range(N_CORES):
        y[core // 4] += res.results[core]["y"].astype(np.float32)
    y += np.asarray(b_proj, np.float32)[None, None, :]
    return y
